# revision 1
# baseline (speedup 1.0000x reference)
"""Trainium2 Bass kernel for CausalSelfAttention (QK-RMSNorm + RoPE).

Sharding: 8 cores = 2 batches x 4 head-groups (4 heads each).
Each core computes QKV projection for its heads, attention, and a partial
output projection (row-parallel c_proj); host sums the 4 partials per batch
and adds b_proj.

Self-contained: hardcodes shapes B=2, T=2048, D=1024, H=16, HD=64.
"""
import os
import sys
import math
from contextlib import ExitStack

for _p in ("/opt/trn_rl_repo", "/root/.axon_site/_ro/trn_rl_repo"):
    if os.path.isdir(_p) and _p not in sys.path:
        sys.path.append(_p)

import numpy as np

import concourse.bass as bass
import concourse.bacc as bacc
import concourse.mybir as mybir
import concourse.tile as tile
from concourse.vector_clock import ScopedClock
from concourse.bass_utils import run_bass_kernel_spmd

B, T, D = 2, 2048, 1024
H, HD = 16, 64
EPS = 1e-6
NCORES = 8
HPC = 4          # heads per core
F = HPC * HD     # 256 features per core per q/k/v
NTB = T // 128   # 16 t-blocks
NIB = T // 512   # 4 i-blocks
F32 = mybir.dt.float32
F32R = mybir.dt.float32r
AF = mybir.ActivationFunctionType
ALU = mybir.AluOpType
AX = mybir.AxisListType


class TileContextSplitDrain(tile.TileContext):
    """Work around walrus 'Too many sync wait commands' on the tail drain:
    split the global-clock waits across single-wait SP drains."""

    MAX_DRAIN_WAITS = 1

    def _drain_and_barrier(self, tick_clock, wait_clock):
        drain_inst = self.nc.sync.drain()
        wait_clock.add_sem_waits(
            drain_inst.ins, ScopedClock({None: tick_clock.global_clock})
        )
        si = drain_inst.ins.sync_info
        waits = list(si.on_wait) if si and si.on_wait else []
        si.on_wait = waits[: self.MAX_DRAIN_WAITS]
        rest = waits[self.MAX_DRAIN_WAITS:]
        while rest:
            d = self.nc.sync.drain()
            d.ins.sync_info = mybir.SyncInfo(
                on_wait=rest[: self.MAX_DRAIN_WAITS], on_update=[]
            )
            rest = rest[self.MAX_DRAIN_WAITS:]

        self.nc.all_engine_barrier()
        assert self.sems is not None
        popped = self.nc._tile_sem_poison_stack.pop()
        assert popped is self._sem_poison
        self.nc.clear_and_free_semaphores(list(self.sems.allocated().values()))
        self.nc.all_engine_barrier()


def r32(ap):
    return ap.bitcast(F32R)


def build_program(exp_bias: float, has_bias: bool = True):
    """One SPMD program; per-core behavior differs only via input data."""
    nc = bacc.Bacc("TRN2", target_bir_lowering=False)
    xt = nc.declare_dram_parameter("xt", [D, T], F32R, isOutput=False)
    wqkv8 = nc.declare_dram_parameter("wqkv8", [128, 8 * 3 * F], F32R, isOutput=False)
    bqkv = nc.declare_dram_parameter("bqkv", [1, 3 * F], F32R, isOutput=False)
    cgqk = nc.declare_dram_parameter("cgqk", [128, NTB * 128], F32, isOutput=False)
    sgqk = nc.declare_dram_parameter("sgqk", [128, NTB * 128], F32, isOutput=False)
    wp2 = nc.declare_dram_parameter("wp2", [128, 2 * D], F32R, isOutput=False)
    trimask = nc.declare_dram_parameter("trimask", [128, 4 * 512], mybir.dt.bfloat16, isOutput=False)
    ident = nc.declare_dram_parameter("ident", [128, 128], F32R, isOutput=False)
    onesd = nc.declare_dram_parameter("onesd", [128, 128], F32R, isOutput=False)
    outp = nc.declare_dram_parameter("outp", [T, D], F32, isOutput=True)

    with tile.TileContext(nc) as tc, ExitStack() as ctx:
        cpool = ctx.enter_context(tc.tile_pool(name="consts", bufs=1))
        big = ctx.enter_context(tc.tile_pool(name="big", bufs=1))
        xtp = ctx.enter_context(tc.tile_pool(name="xtp", bufs=12))
        work = ctx.enter_context(tc.tile_pool(name="work", bufs=3))
        small = ctx.enter_context(tc.tile_pool(name="small", bufs=4))
        epool = ctx.enter_context(tc.tile_pool(name="epool", bufs=3))
        rvp = ctx.enter_context(tc.tile_pool(name="rvp", bufs=3))
        outs = ctx.enter_context(tc.tile_pool(name="outs", bufs=2))
        # PSUM: qkv/tr/pb/pj share 2 two-bank slots (4 banks); S^T pair tiles
        # 1x two-bank slot; PV accumulators 2 one-bank slots.
        psQ = ctx.enter_context(tc.tile_pool(name="psQ", bufs=4, space="PSUM"))
        psS = ctx.enter_context(tc.tile_pool(name="psS", bufs=2, space="PSUM"))
        psO = psQ

        # ---- constants ----
        wqkv_sb = cpool.tile([128, 8, 3 * F], F32R, tag="wqkv")
        wq8v = wqkv8.rearrange("p (k f) -> p k f", k=8)
        nc.sync.dma_start(out=wqkv_sb[:, 0, :], in_=wq8v[:, 0, :])
        bqkv_sb = cpool.tile([1, 3 * F], F32R, tag="bqkv")
        nc.sync.dma_start(out=bqkv_sb, in_=bqkv[:, :])
        cg_sb = cpool.tile([128, NTB, 128], F32, tag="cg")
        sg_sb = cpool.tile([128, NTB, 128], F32, tag="sg")
        wp_sb = cpool.tile([128, 2, D], F32R, tag="wp")
        tri_sb = cpool.tile([128, 4, 512], mybir.dt.bfloat16, tag="tri")
        id_sb = cpool.tile([128, 128], F32R, tag="ident")
        nc.sync.dma_start(out=id_sb, in_=ident[:, :])
        onesP = cpool.tile([128, 128], F32R, tag="onesP")
        nc.sync.dma_start(out=onesP, in_=onesd[:, :])
        eps_b = cpool.tile([128, 1], F32, tag="epsb")
        nc.vector.memset(eps_b, EPS)
        ebias_b = cpool.tile([128, 1], F32, tag="ebiasb")
        nc.vector.memset(ebias_b, float(exp_bias))

        # big persistent tensors
        qkt = big.tile([128, 4, T], F32R, tag="qkt")    # [c, {q0,q1,k0,k1}, t]
        v4 = big.tile([128, NTB, HPC, HD + 1], F32R, tag="v4")  # V|1 [t, head, c]
        ont = big.tile([128, 2, T], F32R, tag="ont")    # normalized O^T chunks

        # V ones-columns (softmax denominator trick), written once
        nc.scalar.copy(
            out=v4[:, :, :, HD:HD + 1],
            in_=onesP[:, 0:NTB * HPC].rearrange("p (t h) -> p t h", t=NTB).unsqueeze(3),
        )

        def emit_A(tb):
            """QKV projection + rmsnorm + rope + transposes for one t-block."""
            ts = slice(tb * 128, (tb + 1) * 128)
            pq = psQ.tile([128, 512], F32, tag="ps", name=f"pq{tb}")
            pv = psQ.tile([128, 256], F32, tag="ps", name=f"pv{tb}")
            xts = []
            for kd in range(8):
                xtile = xtp.tile([128, 128], F32R, tag="xt", name=f"xt{tb}_{kd}")
                nc.sync.dma_start(out=xtile, in_=xt[kd * 128:(kd + 1) * 128, ts])
                xts.append(xtile)
            if tb == 0:
                # deferred constants: behind tb0's xt tiles in the DMA queues,
                # but emitted before their first consumers
                for kd in range(1, 8):
                    nc.sync.dma_start(out=wqkv_sb[:, kd, :], in_=wq8v[:, kd, :])
                nc.scalar.dma_start(
                    out=cg_sb, in_=cgqk.rearrange("p (t c) -> p t c", t=NTB))
                nc.scalar.dma_start(
                    out=sg_sb, in_=sgqk.rearrange("p (t c) -> p t c", t=NTB))
            for kd in range(8):
                nc.tensor.matmul(pq, xts[kd], wqkv_sb[:, kd, 0:512],
                                 start=(kd == 0),
                                 stop=(kd == 7 and not has_bias))
            if has_bias:
                nc.tensor.matmul(pq, onesP[0:1, :], bqkv_sb[:, 0:512],
                                 start=False, stop=True)
            for kd in range(8):
                nc.tensor.matmul(pv, xts[kd], wqkv_sb[:, kd, 512:768],
                                 start=(kd == 0),
                                 stop=(kd == 7 and not has_bias))
            if has_bias:
                nc.tensor.matmul(pv, onesP[0:1, :], bqkv_sb[:, 512:768],
                                 start=False, stop=True)
            # V -> SBUF [t, head, c]
            nc.scalar.copy(
                out=v4[:, tb, :, 0:HD], in_=pv.rearrange("p (h c) -> p h c", h=HPC)
            )
            # stage q|k to SBUF so the PSUM slot frees for the next t-block
            pqs = work.tile([128, 512], F32, tag="pqs", name=f"pqs{tb}")
            nc.scalar.copy(out=pqs, in_=pq)
            # rmsnorm stats over head_dim
            sq = work.tile([128, 512], F32, tag="sq", name=f"sq{tb}")
            nc.vector.tensor_mul(sq, pqs, pqs)
            var = small.tile([128, 8], F32, tag="var", name=f"var{tb}")
            nc.vector.tensor_reduce(
                var, sq.rearrange("p (h c) -> p h c", h=8), AX.X, ALU.add
            )
            rstd_s = small.tile([128, 8], F32, tag="rstds", name=f"rstds{tb}")
            nc.scalar.activation(rstd_s, var, AF.Sqrt, scale=1.0 / HD,
                                 bias=eps_b[:, :])
            rstd = small.tile([128, 8], F32, tag="rstd", name=f"rstd{tb}")
            nc.vector.reciprocal(rstd, rstd_s)
            # qn = q * rstd (per-head broadcast)
            qn = work.tile([128, 512], F32, tag="qn", name=f"qn{tb}")
            nc.vector.tensor_tensor(
                qn.rearrange("p (h c) -> p h c", h=8),
                pqs.rearrange("p (h c) -> p h c", h=8),
                rstd.unsqueeze(2).broadcast_to((128, 8, HD)),
                ALU.mult,
            )
            # rope: qr = qn*CG + shift(qn)*SG
            qn4 = qn.rearrange("p (g h c) -> p g h c", g=2, h=HPC)
            cgs = cg_sb[:, tb, :].rearrange("p (g c) -> p g c", g=2)
            sgs = sg_sb[:, tb, :].rearrange("p (g c) -> p g c", g=2)
            m1 = work.tile([128, 512], F32, tag="m1", name=f"m1_{tb}")
            m1v = m1.rearrange("p (g h c) -> p g h c", g=2, h=HPC)
            nc.vector.tensor_tensor(
                m1v, qn4, cgs.unsqueeze(2).broadcast_to((128, 2, HPC, HD)), ALU.mult
            )
            m2 = work.tile([128, 512], F32, tag="m2", name=f"m2_{tb}")
            m2v = m2.rearrange("p (g h c) -> p g h c", g=2, h=HPC)
            nc.gpsimd.tensor_tensor(
                m2v[:, :, :, 0:32],
                qn4[:, :, :, 32:64],
                sgs[:, :, 0:32].unsqueeze(2).broadcast_to((128, 2, HPC, 32)),
                ALU.mult,
            )
            nc.gpsimd.tensor_tensor(
                m2v[:, :, :, 32:64],
                qn4[:, :, :, 0:32],
                sgs[:, :, 32:64].unsqueeze(2).broadcast_to((128, 2, HPC, 32)),
                ALU.mult,
            )
            qr = work.tile([128, 512], F32R, tag="qr", name=f"qr{tb}")
            nc.vector.tensor_add(qr, m1, m2)
            # transposes -> qkt[:, :, ts]
            tr = psQ.tile([128, 4, 128], F32R, tag="ps", name=f"tr{tb}")
            for cc in range(4):
                nc.tensor.transpose(
                    tr[:, cc, :], qr[:, cc * 128:(cc + 1) * 128], id_sb
                )
            nc.scalar.copy(out=qkt[:, :, ts], in_=tr)

        def emit_B(hp, ib):
            """Attention for one head-pair x query i-block."""
            isl = slice(ib * 512, (ib + 1) * 512)
            njc = 4 * ib + 4
            po = [psO.tile([65, 512], F32, tag="ps", name=f"po{hp}_{ib}_{h}")
                  for h in range(2)]
            for jc in range(njc):
                jsl = slice(jc * 128, (jc + 1) * 128)
                sp = psS.tile([128, 2, 512], F32, tag="sp", name=f"sp{hp}_{ib}_{jc}")
                nc.tensor.matmul(
                    sp[:, 0, :], qkt[0:64, 2 + hp, jsl], qkt[0:64, hp, isl],
                    start=True, stop=True, tile_position=(0, 0),
                )
                nc.tensor.matmul(
                    sp[:, 1, :], qkt[64:128, 2 + hp, jsl], qkt[64:128, hp, isl],
                    start=True, stop=True, tile_position=(64, 0),
                )
                # exp over valid (causal) columns
                s = max(0, 128 * (jc - 4 * ib))
                e = epool.tile([128, 2, 512], F32R, tag="e", name=f"e{hp}_{ib}_{jc}")
                nc.scalar.activation(
                    e[:, :, s:512], sp[:, :, s:512], AF.Exp,
                    scale=1.0 / math.sqrt(HD), bias=ebias_b[:, :],
                )
                if jc >= 4 * ib:  # diagonal-crossing tile: triangle mask
                    r = jc - 4 * ib
                    nc.gpsimd.tensor_mul(
                        e[:, :, s:s + 128],
                        e[:, :, s:s + 128],
                        tri_sb[:, r, s:s + 128].unsqueeze(1).broadcast_to(
                            (128, 2, 128)),
                    )
                first, last = (jc == 0), (jc == njc - 1)
                for h in range(2):
                    head = hp * 2 + h
                    nc.tensor.matmul(
                        po[h][:, s:512], v4[:, jc, head, :], e[:, h, s:512],
                        start=first, stop=last,
                    )
            # normalize: ont[c, i] = po[c, i] / po[64, i]
            rv = rvp.tile([128, 1024], F32R, tag="rv", name=f"rv{hp}_{ib}")
            pos = [rvp.tile([65, 512], F32, tag="pos", name=f"pos{hp}_{ib}_{h}")
                   for h in range(2)]
            stage = outs.tile([64, 512], F32R, tag="stage", name=f"st{hp}_{ib}")
            for h in range(2):
                with nc.allow_low_precision(reason="fp32r softmax denom"):
                    nc.vector.reciprocal(
                        rv[64:65, h * 512:(h + 1) * 512], po[h][64:65, :]
                    )
            nc.scalar.copy(out=pos[0], in_=po[0])
            nc.vector.tensor_copy(pos[1], po[1])
            pb = [psQ.tile([64, 512], F32, tag="ps", name=f"pb{hp}_{ib}_{h}")
                  for h in range(2)]
            for h in range(2):
                nc.tensor.matmul(
                    pb[h], onesP[64:65, 0:64], rv[64:65, h * 512:(h + 1) * 512],
                    start=True, stop=True,
                )
            nc.vector.tensor_mul(ont[0:64, hp, isl], pos[0][0:64, :], pb[0])
            nc.vector.tensor_mul(stage, pos[1][0:64, :], pb[1])
            nc.sync.dma_start(out=ont[64:128, hp, isl], in_=stage)

        def emit_C(tb):
            """Output projection for one t-block (partial; host adds b_proj)."""
            ts = slice(tb * 128, (tb + 1) * 128)
            ob = outs.tile([128, D], F32, tag="ob", name=f"ob{tb}")
            for nh in range(2):
                pj = psQ.tile([128, 512], F32, tag="ps", name=f"pj{tb}_{nh}")
                for hp in range(2):
                    nc.tensor.matmul(
                        pj, ont[:, hp, ts],
                        wp_sb[:, hp, nh * 512:(nh + 1) * 512],
                        start=(hp == 0), stop=(hp == 1),
                    )
                if nh == 0:
                    nc.scalar.copy(out=ob[:, 0:512], in_=pj)
                else:
                    nc.vector.tensor_copy(ob[:, 512:1024], pj)
            nc.sync.dma_start(out=outp[ts, :], in_=ob)

        # ---- interleaved emission: software-pipelined phases ----
        for tb in range(NTB):
            emit_A(tb)
            if tb == 2:  # attention mask + proj weights, before phases B/C
                nc.scalar.dma_start(
                    out=tri_sb, in_=trimask.rearrange("p (r c) -> p r c", r=4))
                nc.scalar.dma_start(
                    out=wp_sb, in_=wp2.rearrange("p (k f) -> p k f", k=2))
        for ib in range(NIB):
            for hp in range(2):
                emit_B(hp, ib)
            if ib > 0:
                for tb in range(4 * (ib - 1), 4 * ib):
                    emit_C(tb)
        for tb in range(12, 16):
            emit_C(tb)

    nc.compile()
    return nc


def host_inputs(x, w_attn, b_attn, w_proj, g_q, g_k, rope_cos, rope_sin):
    """Per-core input maps + exp bias."""
    x = np.asarray(x, dtype=np.float32)
    w_attn = np.asarray(w_attn, dtype=np.float32)
    b_attn = np.asarray(b_attn, dtype=np.float32)
    w_proj = np.asarray(w_proj, dtype=np.float32)
    g_q = np.asarray(g_q, dtype=np.float32)
    g_k = np.asarray(g_k, dtype=np.float32)
    rope_cos = np.asarray(rope_cos, dtype=np.float32)
    rope_sin = np.asarray(rope_sin, dtype=np.float32)

    # |s| <= 8 * max|g_q| * max|g_k| after RMSNorm; subtract for exp safety
    bound = 8.0 * max(1e-6, float(np.abs(g_q).max())) * max(
        1e-6, float(np.abs(g_k).max())
    )
    exp_bias = -bound

    # rope tables with gains folded in; shifted-sign sin for rotate_half
    def sg_of(g):
        sgn = np.where(np.arange(HD) < HD // 2, -1.0, 1.0).astype(np.float32)
        gperm = np.roll(g, HD // 2)  # g[(c+32)%64]
        return rope_sin * (sgn * gperm)[None, :]  # [T, HD]

    cgq = rope_cos * g_q[None, :]
    cgk = rope_cos * g_k[None, :]
    sgq = sg_of(g_q)
    sgk = sg_of(g_k)

    def arrange_rope(a_q, a_k):
        # [T, HD] x2 -> [128, NTB*128] with [p, tb, {q:64 | k:64}]
        aq = a_q.reshape(NTB, 128, HD).transpose(1, 0, 2)
        ak = a_k.reshape(NTB, 128, HD).transpose(1, 0, 2)
        return np.ascontiguousarray(
            np.concatenate([aq, ak], axis=2).reshape(128, NTB * 128)
        )

    cg_arr = arrange_rope(cgq, cgk)
    sg_arr = arrange_rope(sgq, sgk)

    # masks[j, r, :]: zeros for cols < 128r, triu(j <= i') on cols [128r,128r+128)
    tri = np.zeros((128, 4, 512), dtype=np.float32)
    for r in range(4):
        tri[:, r, 128 * r:128 * (r + 1)] = np.triu(np.ones((128, 128), np.float32))
        tri[:, r, 128 * (r + 1):] = 1.0
    import ml_dtypes
    tri = np.ascontiguousarray(tri.reshape(128, 4 * 512)).astype(ml_dtypes.bfloat16)
    ident = np.eye(128, dtype=np.float32)

    in_maps = []
    for c in range(NCORES):
        b, hg = divmod(c, 4)
        f0 = hg * F
        rows = np.concatenate([
            np.arange(f0, f0 + F),
            D + np.arange(f0, f0 + F),
            2 * D + np.arange(f0, f0 + F),
        ])
        w = w_attn[rows]                      # [768, 1024]
        wqkvT = np.ascontiguousarray(w.T)     # [1024, 768]
        wqkv8 = np.ascontiguousarray(
            wqkvT.reshape(8, 128, 3 * F).transpose(1, 0, 2).reshape(128, 8 * 3 * F)
        )
        bq = np.ascontiguousarray(b_attn[rows].reshape(1, 3 * F))
        wpT = np.ascontiguousarray(w_proj[:, f0:f0 + F].T)  # [256, 1024]
        wp2 = np.ascontiguousarray(
            wpT.reshape(2, 128, D).transpose(1, 0, 2).reshape(128, 2 * D)
        )
        in_maps.append({
            "xt": np.ascontiguousarray(x[b].T),
            "wqkv8": wqkv8,
            "bqkv": bq,
            "cgqk": cg_arr,
            "sgqk": sg_arr,
            "wp2": wp2,
            "trimask": tri,
            "onesd": np.ones((128, 128), dtype=np.float32),
            "ident": ident,
        })
    return in_maps, exp_bias


_CACHE = {}


def kernel(x, w_attn, b_attn, w_proj, b_proj, g_q, g_k, rope_cos, rope_sin):
    in_maps, exp_bias = host_inputs(
        x, w_attn, b_attn, w_proj, g_q, g_k, rope_cos, rope_sin
    )
    has_bias = bool(np.any(np.asarray(b_attn)))
    key = (float(exp_bias), has_bias)
    if key not in _CACHE:
        _CACHE[key] = build_program(exp_bias, has_bias)
    nc = _CACHE[key]
    res = run_bass_kernel_spmd(nc, in_maps, list(range(NCORES)))
    out = np.zeros((B, T, D), dtype=np.float32)
    for c in range(NCORES):
        out[c // 4] += res.results[c]["outp"]
    out += np.asarray(b_proj, dtype=np.float32)[None, None, :]
    return out

